# revision 2
# baseline (speedup 1.0000x reference)
"""Trainium2 Bass kernel for per-voxel 3x3 SPD matrix logarithm (v4).

Input  x: (2, 9, 64, 128, 128) fp32, channel c = 3*i+j of symmetric M.
Output Y: same shape, Y = U log(S) U^T per voxel.

Design:
  - bf16 end-to-end: host converts input to bf16 (6 unique channels), device
    computes mostly in bf16 (DVE 2x_1p fast mode), writes bf16 output
    (6 unique channels); host upcasts + mirrors symmetric entries.
  - stable divided differences in bf16: c1 = -ln(1 - d12/l2)/d12,
    c1' = ln(1 + d23/l2)/d23 (log1p-form); gamma = uu*cc2*(sigma-uu) + ln(l1).
  - custom DVE ops fuse reciprocal seeds (BITWISE_NOT + Newton).
  - 4 fully independent chunk pipelines (per-chunk tile tags, heavy tile
    multi-use keeps SBUF under budget); ACT phases batched across all four
    chunks via value-correct gated bias tiles -> 4 table loads total.

Tile multi-use map (per chunk):
  xin[0:6]: input a,d,f,b,c,e  ->  final output Yd(3)|Yo(3)
  SQ[0:6]:  D^2(3)|b2,c2,e2(3) -> pair sums(lane0,1) -> tau(0:3),
            dets(lane1) -> LD [l1|l3|l2|d12|d23|d13] -> zd scratch(0:3),
            o1 scratch(3:6)
  ip(F32):  1/p -> rr -> at;  r2v(F32): r^2 -> tq;  sqv(F32): sqrt(1-r^2)
  sfcf:     sin/cos -> ps3, pc2;  G2: g12,g23 -> s0, sigma;  cf2: c1n,c1b -> e1,e2
"""
import math
import numpy as np
import ml_dtypes

import concourse.bacc as bacc
import concourse.tile as tile
import concourse.bass as bass
from concourse import mybir
from concourse.bass_utils import run_bass_kernel_spmd

F32 = mybir.dt.float32
BF16 = mybir.dt.bfloat16
OP = mybir.AluOpType
AF = mybir.ActivationFunctionType

B = 2
NV = 64 * 128 * 128
NCORE = 8
VPC = NV // NCORE
P = 128
FD = 512
CPB = VPC // (P * FD)        # 2 chunks per batch
NCHUNK = B * CPB             # 4
PLANE = VPC // P
F6 = 6 * FD

CL = 0.99999988
S3 = math.sqrt(3.0)
PI6 = math.pi / 6.0
# Gap clamp: divided differences are evaluated over gaps >= TINY. The ACT Ln
# LUT has ~6e-8 absolute noise; c1 error ~ noise/gap, so 1e-3 keeps that at
# ~6e-5 while the clamp bias (f'' * gap / 2) is ~1e-4 — both negligible.
TINY = 1e-3

# ---- runtime-registered custom DVE ops ----
from concourse import dve_ops as _dvo
from concourse.dve_spec import (
    Spec as _Spec, Src0 as _S0, Src1 as _S1, C0 as _C0, C1 as _C1, C2 as _C2,
    maxx as _maxx, minn as _minn, lower as _lower, _has_src1 as _hs1, Bin as _Bin,
    AluOp as _AluOp,
)
from concourse.dve_uop import DveOpSpec as _DveOpSpec


def _register_dve(name, spec):
    if name in _dvo._SUB_OPCODE_FOR_NAME:
        return next(op for op in _dvo.OPS if op.name == name)
    op = _dvo.DveOp(name, spec, subdim=False, uops_sha={})
    _dvo.OPS.append(op)
    _dvo.CUSTOM_DVE_SPECS[name] = spec
    row = _dvo._CUSTOM_DVE_ROW_BASE + len(_dvo.OPS) - 1
    assert row < 0x20
    _dvo._SUB_OPCODE_FOR_NAME[name] = row
    for ver in ("v3", "v4"):
        uops = _lower(spec, ver=ver)
        res = _DveOpSpec(name=name, opcode=row, uops=uops, rd1_en=_hs1(spec))
        op.uops_sha[ver] = res.sha(ver)
    return op


_not0 = _Bin(_AluOp.BITWISE_NOT, _S0, _S0)
_ry0 = _not0 * _C0
_ry1 = _ry0 * (_C1 - _S0 * _ry0)
_RC0 = -0.23549792
_RC1 = 2.0017324


def _np_recip1(x):
    x = np.asarray(x, np.float32)
    y0 = (~x.view(np.int32)).view(np.float32) * np.float32(_RC0)
    return (y0 * (np.float32(_RC1) - x * y0)).astype(np.float32)


RECIP1 = _register_dve("LOGM_RECIP1", _Spec(
    body=_ry1,
    reference=lambda in0, in1, s0, s1, imm2: _np_recip1(in0),
))
RECIP1_MUL = _register_dve("LOGM_RECIP1_MUL", _Spec(
    body=_ry1 * _S1,
    reference=lambda in0, in1, s0, s1, imm2: (
        _np_recip1(in0) * np.asarray(in1, np.float32)).astype(np.float32),
))
DETC_CLAMP = _register_dve("LOGM_DETC_CLAMP", _Spec(
    body=_minn(_maxx(_S0 * (_S1 * _S1 * _S1) * _C0, _C1), _C2),
    reference=lambda in0, in1, s0, s1, imm2: np.minimum(
        np.maximum(np.asarray(in0, np.float32) * (np.asarray(in1, np.float32) ** 3) * s0, s1),
        imm2).astype(np.float32),
))
SCALE_SUBMAX = _register_dve("LOGM_SCALE_SUBMAX", _Spec(
    body=_maxx(_S0 * _C0 - _S1, _C1),
    reference=lambda in0, in1, s0, s1, imm2: np.maximum(
        np.asarray(in0, np.float32) * s0 - np.asarray(in1, np.float32), s1
    ).astype(np.float32),
))

# Force Arctan into trig_and_small so a chunk's trig phase is one table load.
from concourse import hw_specs as _hw
import concourse.bacc as _bacc_mod
_orig_gat = _hw.get_activation_tables


def _patched_gat(arch):
    t = _orig_gat(arch)
    for sname, fns in t.items():
        if sname != "trig_and_small":
            fns.discard(mybir.ActivationFunctionType.Arctan)
    return t


_hw.get_activation_tables = _patched_gat
_bacc_mod.get_activation_tables = _patched_gat

_CACHE = {}


def _register_const(nc, val):
    t = nc.alloc_sbuf_tensor(f"const-f32-{val}", [128, 1], F32)
    nc.gpsimd.memset(t.ap(), val)
    nc.const_aps.aps[(F32, float(val))] = t.ap()


def _b(ap_fd, n):
    return ap_fd.unsqueeze(1).broadcast_to((P, n, FD))


def build():
    nc = bacc.Bacc("TRN2")
    _register_const(nc, 1.0)
    _register_const(nc, 1e-30)
    _register_const(nc, PI6)
    _register_const(nc, PI6 + math.pi / 2.0)
    nc.all_engine_barrier()

    xin = nc.dram_tensor("xin", [NCHUNK, P, F6], BF16, kind="ExternalInput")
    yout = nc.dram_tensor("yout", [NCHUNK, P, F6], BF16, kind="ExternalOutput")

    V, G, S = nc.vector, nc.gpsimd, nc.scalar

    with tile.TileContext(nc) as tc:
        with tc.tile_pool(name="mp", bufs=1) as pool:

            def T(units, name, ci, dtype=BF16):
                tag = f"{name}{ci}"
                return pool.tile([P, units * FD], dtype, name=tag, tag=tag, bufs=1)

            def r3(ap):
                return ap.rearrange("p (c f) -> p c f", c=3)

            def r2_(ap):
                return ap.rearrange("p (c f) -> p c f", c=2)

            def mk_bias(name, probes, vals):
                """[P,1] bias-constant tiles whose writes depend on `probes`.

                Values are exact; the dependency orders a phase's ACT ops
                after all probes' producers, so the activation-table fixpoint
                sees clean one-table runs."""
                pb = pool.tile([P, 1], F32, name=f"pb_{name}", tag=f"pb_{name}")
                V.tensor_tensor(pb[:], probes[0], probes[1], OP.add)
                for pr in probes[2:]:
                    V.tensor_tensor(pb[:], pb[:], pr, OP.add)
                bt = pool.tile([P, len(vals)], F32, name=f"bt_{name}", tag=f"bt_{name}")
                out = {}
                for i, v in enumerate(vals):
                    V.tensor_scalar(bt[:, i:i + 1], pb[:], 0.0, float(v),
                                    OP.mult, OP.add)
                    out[v] = bt[:, i:i + 1]
                return out

            def phaseA(ci):
                t = {}
                xin_t = T(6, "xin", ci)
                t["xin"] = xin_t
                if ci == 0:
                    for u0, u1 in ((0, 2), (2, 4), (4, 6)):
                        srcp = bass.AP(xin, ci * P * F6 + u0 * FD,
                                       [[F6, P], [1, (u1 - u0) * FD]])
                        nc.sync.dma_start(xin_t[:, u0 * FD:u1 * FD], srcp)
                else:
                    src_adf = bass.AP(xin, ci * P * F6, [[F6, P], [1, 3 * FD]])
                    src_bce = bass.AP(xin, ci * P * F6 + 3 * FD, [[F6, P], [1, 3 * FD]])
                    nc.sync.dma_start(xin_t[:, 0:3 * FD], src_adf)
                    nc.sync.dma_start(xin_t[:, 3 * FD:6 * FD], src_bce)
                xr = xin_t[:]
                b_ = xr[:, 3 * FD:4 * FD]
                c_ = xr[:, 4 * FD:5 * FD]
                e_ = xr[:, 5 * FD:6 * FD]
                adf = r3(xr[:, 0:3 * FD])
                bce = xr[:, 3 * FD:6 * FD]

                q = T(1, "q", ci)
                V.tensor_tensor(q[:], xr[:, 0:FD], xr[:, FD:2 * FD], OP.add)
                V.tensor_tensor(q[:], q[:], xr[:, 2 * FD:3 * FD], OP.add)
                V.tensor_scalar(q[:], q[:], 1.0 / 3.0, None, OP.mult)
                t["q"] = q
                D3 = T(3, "D3", ci)
                V.tensor_tensor(r3(D3[:]), adf, _b(q[:], 3), OP.subtract)
                t["D3"] = D3

                SQ = T(6, "SQ", ci)
                S.activation(SQ[:, 0:3 * FD], D3[:], AF.Square)
                S.activation(SQ[:, 3 * FD:6 * FD], bce, AF.Square)
                t["SQ"] = SQ
                sq6 = SQ[:].rearrange("p (c f) -> p c f", c=6)
                sqo_rev = r3(SQ[:, 3 * FD:6 * FD])[:, ::-1, :]

                # pair-reduce into SQ lanes [0,1] (D^2 lanes, dead after st):
                # (D2a,b2)+(D2d,c2) -> (0,1); + (D2f,e2) -> stsu
                V.tensor_tensor(sq6[:, 0:2, :], sq6[:, 0:4:3, :], sq6[:, 1:5:3, :], OP.add)
                stsu = T(2, "stsu", ci)
                V.tensor_tensor(r2_(stsu[:]), sq6[:, 0:2, :], sq6[:, 2:6:3, :], OP.add)
                st = stsu[:, 0:FD]
                su = stsu[:, FD:2 * FD]
                t["su"] = stsu
                # p2u = st + 2*su accumulated in-place into the st lane
                V.tensor_tensor(st, st, su, OP.add)
                V.tensor_tensor(st, st, su, OP.add)
                pt = T(1, "pt", ci)
                S.activation(pt[:], st, AF.Sqrt, scale=1.0 / 6.0, bias=1e-30)
                t["pt"] = pt

                # det block: tau into SQ[0:3] (D^2 lanes dead after stsu)
                tau = SQ[:, 0:3 * FD]
                V.tensor_tensor(r3(tau), r3(D3[:]), sqo_rev, OP.mult)
                V.tensor_tensor(SQ[:, 0:FD], SQ[:, 0:FD], SQ[:, FD:2 * FD], OP.add)
                V.tensor_tensor(SQ[:, FD:2 * FD], SQ[:, 0:FD], SQ[:, 2 * FD:3 * FD], OP.add)
                dets = SQ[:, FD:2 * FD]
                ad = T(1, "ad", ci)
                V.tensor_tensor(ad[:], D3[:, 0:FD], D3[:, FD:2 * FD], OP.mult)
                V.tensor_tensor(ad[:], ad[:], D3[:, 2 * FD:3 * FD], OP.mult)
                cross = T(3, "cross", ci)
                t["cross"] = cross
                V.tensor_tensor(cross[:, 2 * FD:3 * FD], b_, c_, OP.mult)
                bce2 = T(1, "bce2", ci)
                V.tensor_tensor(bce2[:], cross[:, 2 * FD:3 * FD], e_, OP.mult)
                V.tensor_scalar(bce2[:], bce2[:], 2.0, None, OP.mult)
                V.tensor_tensor(ad[:], ad[:], dets, OP.subtract)
                V.tensor_tensor(ad[:], ad[:], bce2[:], OP.add)   # ad holds det
                t["det"] = ad

                # Pool: cross01 = (c,b)*e ; w3 = su - (e2,c2,b2)
                cb = r3(xr[:, 3 * FD:6 * FD])[:, 1::-1, :]
                G.tensor_tensor(r2_(cross[:, 0:2 * FD]), cb,
                                e_.unsqueeze(1).broadcast_to((P, 2, FD)), OP.mult)
                w3 = T(3, "w3", ci)
                G.tensor_tensor(r3(w3[:]), _b(su, 3), sqo_rev, OP.subtract)
                t["w3"] = w3
                return t

            def phaseAtail(ci, t, gate=None):
                gb = gate if gate is not None else {}
                ip = T(1, "ip", ci, F32)
                from concourse.dve_ops import (RECIP_APPROX_FAST_CONSTS as _RC,
                                               RECIPROCAL_APPROX_FAST as _RAF)
                V._custom_dve(_RAF, out=ip[:], in0=t["pt"][:],
                              s0=_RC["s0"], s1=_RC["s1"], imm2=_RC["imm2"])
                # rr overwrites ip (in1 of the custom; elementwise-safe)
                V._custom_dve(DETC_CLAMP, out=ip[:], in0=t["det"][:], in1=ip[:],
                              s0=0.5, s1=-CL, imm2=CL)
                rr = ip
                r2v = T(1, "r2v", ci, F32)
                S.activation(r2v[:], rr[:], AF.Square)
                sqv = T(1, "sqv", ci, F32)
                S.activation(sqv[:], r2v[:], AF.Sqrt, scale=-1.0, bias=gb.get(1.0, 1.0))
                # tq -> overwrite r2v (dead)
                V._custom_dve(RECIP1_MUL, out=r2v[:], in0=sqv[:], in1=rr[:],
                              s0=_RC0, s1=_RC1)
                t["tq"] = r2v
                t["sqv"] = sqv
                t["at"] = rr   # rr dead after tq; reuse for arctan output

            def phaseB1(ci, t, gate=None):
                gb = gate if gate is not None else {}
                at = t["at"]
                S.activation(at[:], t["tq"][:], AF.Arctan, bias=gb.get(0.0, 0.0))
                sfcf = T(2, "sfcf", ci)
                S.activation(sfcf[:, 0:FD], at[:], AF.Sin, scale=-1.0 / 3.0,
                             bias=gb.get(PI6, PI6))
                S.activation(sfcf[:, FD:2 * FD], at[:], AF.Sin, scale=-1.0 / 3.0,
                             bias=gb.get(PI6 + math.pi / 2.0, PI6 + math.pi / 2.0))
                t["sfcf"] = sfcf
                pcps = T(2, "pcps", ci)
                V.tensor_tensor(r2_(pcps[:]),
                                t["pt"][:].unsqueeze(1).broadcast_to((P, 2, FD)),
                                r2_(sfcf[:]), OP.mult)
                ps = pcps[:, 0:FD]
                pc = pcps[:, FD:2 * FD]
                t["pcps"] = pcps
                # sfcf dead: reuse lanes for ps3, pc2
                ps3 = sfcf[:, 0:FD]
                pc2 = sfcf[:, FD:2 * FD]
                V.tensor_scalar(ps3, ps, S3, None, OP.mult)
                V.tensor_scalar(pc2, pc, 2.0, None, OP.mult)
                t["pc2h"] = sfcf
                uu = T(1, "uu", ci)
                V.tensor_tensor(uu[:], ps3, pc, OP.add)
                t["uu"] = uu

                # LD lanes into SQ: [l1 | l3 | l2 | d12 | d23 | d13]
                SQ = t["SQ"]
                LD = SQ[:]
                ld6 = LD.rearrange("p (c f) -> p c f", c=6)
                V.tensor_tensor(LD[:, 0:FD], t["q"][:], uu[:], OP.subtract)      # l1
                V.tensor_tensor(LD[:, FD:2 * FD], t["q"][:], pc2, OP.add)        # l3
                V.tensor_scalar(LD[:, 3 * FD:4 * FD], ps3, 2.0, TINY,
                                OP.mult, OP.max)                                 # d12
                V._custom_dve(SCALE_SUBMAX, out=LD[:, 4 * FD:5 * FD], in0=pc,
                              in1=uu[:], s0=4.0, s1=TINY)                        # d23
                # (l2, d13) = (l1, d12) + (d12, d23)
                V.tensor_tensor(ld6[:, 2:6:3, :], ld6[:, 0:4:3, :],
                                ld6[:, 3:5:1, :], OP.add)
                RL = T(4, "RL", ci)
                V._custom_dve(RECIP1, out=RL[:].rearrange("p (c f) -> p c f", c=4),
                              in0=LD[:, 2 * FD:6 * FD].rearrange("p (c f) -> p c f", c=4),
                              s0=_RC0, s1=_RC1)
                t["RL"] = RL
                U2 = T(2, "U2", ci)
                V.tensor_tensor(r2_(U2[:]), ld6[:, 3:5, :],
                                RL[:, 0:FD].unsqueeze(1).broadcast_to((P, 2, FD)),
                                OP.mult)
                t["U2"] = U2

            def phaseB2(ci, t, gate=None):
                gb = gate if gate is not None else {}
                G2 = T(2, "G2", ci)
                S.activation(G2[:, 0:FD], t["U2"][:, 0:FD], AF.Ln, scale=-1.0,
                             bias=gb.get(1.0, 1.0))
                S.activation(G2[:, FD:2 * FD], t["U2"][:, FD:2 * FD], AF.Ln,
                             bias=gb.get(1.0, 1.0))
                lg1 = T(1, "lg1", ci)
                S.activation(lg1[:], t["SQ"][:, 0:FD], AF.Ln, bias=gb.get(0.0, 0.0))
                t["lg1"] = lg1
                cf2 = T(2, "cf2", ci)
                V.tensor_tensor(r2_(cf2[:]), r2_(G2[:]),
                                t["RL"][:, FD:3 * FD].rearrange("p (c f) -> p c f", c=2),
                                OP.mult)
                cc2 = T(1, "cc2", ci)
                V.tensor_tensor(cc2[:], cf2[:, 0:FD], cf2[:, FD:2 * FD], OP.add)
                V.tensor_tensor(cc2[:], cc2[:], t["RL"][:, 3 * FD:4 * FD], OP.mult)
                # s0 -> G2[0] (dead), sigma -> G2[1]
                s0_ = G2[:, 0:FD]
                sigma = G2[:, FD:2 * FD]
                V._custom_dve(RECIP1_MUL, out=s0_, in0=cc2[:], in1=cf2[:, 0:FD],
                              s0=_RC0, s1=_RC1)
                pc2 = t["pc2h"][:, FD:2 * FD]
                V.tensor_tensor(sigma, pc2, s0_, OP.subtract)
                uu = t["uu"]
                # gamma = uu*cc2*(sigma-uu) + lg1  (e1 -> cf2[0], e2 -> cf2[1])
                e1 = cf2[:, 0:FD]
                e2 = cf2[:, FD:2 * FD]
                V.tensor_tensor(e1, sigma, uu[:], OP.subtract)
                V.tensor_tensor(e2, uu[:], cc2[:], OP.mult)
                V.tensor_tensor(e1, e1, e2, OP.mult)
                V.tensor_tensor(lg1[:], e1, lg1[:], OP.add)
                gamma = lg1
                t["cc2"] = cc2
                t["sigma"] = sigma
                t["gamma"] = gamma

            def phaseB2out(ci, t):
                cc2 = t["cc2"]
                sigma = t["sigma"]
                gamma = t["gamma"]
                D3 = t["D3"]
                SQ = t["SQ"]
                xr = t["xin"][:]
                # diag: zd scratch in SQ[0:3] (l1 consumed by lg1 already)
                zd = SQ[:, 0:3 * FD]
                V.tensor_tensor(r3(zd), r3(D3[:]), _b(sigma, 3), OP.add)
                V.tensor_tensor(zd, zd, D3[:], OP.mult)
                V.tensor_tensor(zd, zd, t["w3"][:], OP.add)
                # Yd into xin[0:3] (adf dead), gamma-add in place, then DMA
                yd = xr[:, 0:3 * FD]
                V.tensor_tensor(r3(yd), r3(zd), _b(cc2[:], 3), OP.mult)
                V.tensor_tensor(r3(yd), r3(yd), _b(gamma[:], 3), OP.add)
                dst_d = bass.AP(yout, ci * P * F6, [[F6, P], [1, 3 * FD]])
                nc.sync.dma_start(dst_d, yd)

                # offdiag: o chain in SQ[3:6] (gap lanes dead after RL/U2);
                # Pool normally, V for the final chunk to shorten the drain
                E = V if ci == NCHUNK - 1 else G
                bce = xr[:, 3 * FD:6 * FD]
                zo = SQ[:, 3 * FD:6 * FD]
                E.tensor_tensor(r3(zo), _b(sigma, 3), r3(D3[:])[:, ::-1, :],
                                OP.subtract)
                E.tensor_tensor(zo, bce, zo, OP.mult)
                E.tensor_tensor(zo, zo, t["cross"][:], OP.add)
                yo = xr[:, 3 * FD:6 * FD]
                V.tensor_tensor(r3(yo), r3(zo), _b(cc2[:], 3), OP.mult)
                dst_o = bass.AP(yout, ci * P * F6 + 3 * FD, [[F6, P], [1, 3 * FD]])
                nc.sync.dma_start(dst_o, yo)

            ts = [phaseA(ci) for ci in range(NCHUNK)]
            for ci in range(NCHUNK):
                phaseAtail(ci, ts[ci])
            g_trig = mk_bias("trig", [ts[ci]["sqv"][:, 0:1] for ci in range(NCHUNK)],
                             [0.0, PI6, PI6 + math.pi / 2.0])
            for ci in range(NCHUNK):
                phaseB1(ci, ts[ci], g_trig)
            g_ln = mk_bias("ln", [ts[ci]["sfcf"][:, 0:1] for ci in range(NCHUNK)],
                           [1.0, 0.0])
            for ci in range(NCHUNK):
                phaseB2(ci, ts[ci], g_ln)
            for ci in range(NCHUNK):
                phaseB2out(ci, ts[ci])
    nc.finalize()
    return nc


def kernel(x):
    x = np.ascontiguousarray(np.asarray(x), dtype=np.float32)
    xf = x.reshape(B, 9, NV)
    sel = [0, 4, 8, 1, 2, 5]  # a d f b c e
    in_maps = []
    for k in range(NCORE):
        sh = xf[:, sel, k * VPC:(k + 1) * VPC]
        sh = sh.reshape(B, 6, P, CPB, FD).transpose(0, 3, 2, 1, 4)
        arr = np.ascontiguousarray(sh).astype(ml_dtypes.bfloat16)
        in_maps.append({"xin": arr.reshape(NCHUNK, P, F6)})
    if "nc" not in _CACHE:
        _CACHE["nc"] = build()
    res = run_bass_kernel_spmd(_CACHE["nc"], in_maps, core_ids=list(range(NCORE)))
    out = np.empty((B, 9, NV), np.float32)
    for k in range(NCORE):
        yb = np.asarray(res.results[k]["yout"]).reshape(B, CPB, P, 6, FD)
        y6 = yb.astype(np.float32).transpose(0, 3, 2, 1, 4).reshape(B, 6, VPC)
        sl = slice(k * VPC, (k + 1) * VPC)
        out[:, 0, sl] = y6[:, 0]
        out[:, 4, sl] = y6[:, 1]
        out[:, 8, sl] = y6[:, 2]
        out[:, 1, sl] = y6[:, 3]
        out[:, 3, sl] = y6[:, 3]
        out[:, 2, sl] = y6[:, 4]
        out[:, 6, sl] = y6[:, 4]
        out[:, 5, sl] = y6[:, 5]
        out[:, 7, sl] = y6[:, 5]
    return out.reshape(x.shape)


# revision 3
# speedup vs baseline: 1.0041x; 1.0041x over previous
"""Trainium2 Bass kernel for per-voxel 3x3 SPD matrix logarithm (v4).

Input  x: (2, 9, 64, 128, 128) fp32, channel c = 3*i+j of symmetric M.
Output Y: same shape, Y = U log(S) U^T per voxel.

Design:
  - bf16 end-to-end: host converts input to bf16 (6 unique channels), device
    computes mostly in bf16 (DVE 2x_1p fast mode), writes bf16 output
    (6 unique channels); host upcasts + mirrors symmetric entries.
  - stable divided differences in bf16: c1 = -ln(1 - d12/l2)/d12,
    c1' = ln(1 + d23/l2)/d23 (log1p-form); gamma = uu*cc2*(sigma-uu) + ln(l1).
  - custom DVE ops fuse reciprocal seeds (BITWISE_NOT + Newton).
  - 4 fully independent chunk pipelines (per-chunk tile tags, heavy tile
    multi-use keeps SBUF under budget); ACT phases batched across all four
    chunks via value-correct gated bias tiles -> 4 table loads total.

Tile multi-use map (per chunk):
  xin[0:6]: input a,d,f,b,c,e  ->  final output Yd(3)|Yo(3)
  SQ[0:6]:  D^2(3)|b2,c2,e2(3) -> pair sums(lane0,1) -> tau(0:3),
            dets(lane1) -> LD [l1|l3|l2|d12|d23|d13] -> zd scratch(0:3),
            o1 scratch(3:6)
  ip(F32):  1/p -> rr -> at;  r2v(F32): r^2 -> tq;  sqv(F32): sqrt(1-r^2)
  sfcf:     sin/cos -> ps3, pc2;  G2: g12,g23 -> s0, sigma;  cf2: c1n,c1b -> e1,e2
"""
import math
import numpy as np
import ml_dtypes

import concourse.bacc as bacc
import concourse.tile as tile
import concourse.bass as bass
from concourse import mybir
from concourse.bass_utils import run_bass_kernel_spmd

F32 = mybir.dt.float32
BF16 = mybir.dt.bfloat16
OP = mybir.AluOpType
AF = mybir.ActivationFunctionType

B = 2
NV = 64 * 128 * 128
NCORE = 8
VPC = NV // NCORE
P = 128
FD = 512
CPB = VPC // (P * FD)        # 2 chunks per batch
NCHUNK = B * CPB             # 4
PLANE = VPC // P
F6 = 6 * FD

CL = 0.99999988
S3 = math.sqrt(3.0)
PI6 = math.pi / 6.0
# Gap clamp: divided differences are evaluated over gaps >= TINY. The ACT Ln
# LUT has ~6e-8 absolute noise; c1 error ~ noise/gap, so 1e-3 keeps that at
# ~6e-5 while the clamp bias (f'' * gap / 2) is ~1e-4 — both negligible.
TINY = 1e-3

# ---- runtime-registered custom DVE ops ----
from concourse import dve_ops as _dvo
from concourse.dve_spec import (
    Spec as _Spec, Src0 as _S0, Src1 as _S1, C0 as _C0, C1 as _C1, C2 as _C2,
    maxx as _maxx, minn as _minn, lower as _lower, _has_src1 as _hs1, Bin as _Bin,
    AluOp as _AluOp,
)
from concourse.dve_uop import DveOpSpec as _DveOpSpec


def _register_dve(name, spec):
    if name in _dvo._SUB_OPCODE_FOR_NAME:
        return next(op for op in _dvo.OPS if op.name == name)
    op = _dvo.DveOp(name, spec, subdim=False, uops_sha={})
    _dvo.OPS.append(op)
    _dvo.CUSTOM_DVE_SPECS[name] = spec
    row = _dvo._CUSTOM_DVE_ROW_BASE + len(_dvo.OPS) - 1
    assert row < 0x20
    _dvo._SUB_OPCODE_FOR_NAME[name] = row
    for ver in ("v3", "v4"):
        uops = _lower(spec, ver=ver)
        res = _DveOpSpec(name=name, opcode=row, uops=uops, rd1_en=_hs1(spec))
        op.uops_sha[ver] = res.sha(ver)
    return op


_not0 = _Bin(_AluOp.BITWISE_NOT, _S0, _S0)
_ry0 = _not0 * _C0
_ry1 = _ry0 * (_C1 - _S0 * _ry0)
_RC0 = -0.23549792
_RC1 = 2.0017324


def _np_recip1(x):
    x = np.asarray(x, np.float32)
    y0 = (~x.view(np.int32)).view(np.float32) * np.float32(_RC0)
    return (y0 * (np.float32(_RC1) - x * y0)).astype(np.float32)


RECIP1 = _register_dve("LOGM_RECIP1", _Spec(
    body=_ry1,
    reference=lambda in0, in1, s0, s1, imm2: _np_recip1(in0),
))
RECIP1_MUL = _register_dve("LOGM_RECIP1_MUL", _Spec(
    body=_ry1 * _S1,
    reference=lambda in0, in1, s0, s1, imm2: (
        _np_recip1(in0) * np.asarray(in1, np.float32)).astype(np.float32),
))
DETC_CLAMP = _register_dve("LOGM_DETC_CLAMP", _Spec(
    body=_minn(_maxx(_S0 * (_S1 * _S1 * _S1) * _C0, _C1), _C2),
    reference=lambda in0, in1, s0, s1, imm2: np.minimum(
        np.maximum(np.asarray(in0, np.float32) * (np.asarray(in1, np.float32) ** 3) * s0, s1),
        imm2).astype(np.float32),
))
SCALE_SUBMAX = _register_dve("LOGM_SCALE_SUBMAX", _Spec(
    body=_maxx(_S0 * _C0 - _S1, _C1),
    reference=lambda in0, in1, s0, s1, imm2: np.maximum(
        np.asarray(in0, np.float32) * s0 - np.asarray(in1, np.float32), s1
    ).astype(np.float32),
))

# Force Arctan into trig_and_small so a chunk's trig phase is one table load.
from concourse import hw_specs as _hw
import concourse.bacc as _bacc_mod
_orig_gat = _hw.get_activation_tables


def _patched_gat(arch):
    t = _orig_gat(arch)
    for sname, fns in t.items():
        if sname != "trig_and_small":
            fns.discard(mybir.ActivationFunctionType.Arctan)
    return t


_hw.get_activation_tables = _patched_gat
_bacc_mod.get_activation_tables = _patched_gat

_CACHE = {}


def _register_const(nc, val):
    t = nc.alloc_sbuf_tensor(f"const-f32-{val}", [128, 1], F32)
    nc.gpsimd.memset(t.ap(), val)
    nc.const_aps.aps[(F32, float(val))] = t.ap()


def _b(ap_fd, n):
    return ap_fd.unsqueeze(1).broadcast_to((P, n, FD))


def build():
    nc = bacc.Bacc("TRN2")
    _register_const(nc, 1.0)
    _register_const(nc, 1e-30)
    _register_const(nc, PI6)
    _register_const(nc, PI6 + math.pi / 2.0)
    nc.all_engine_barrier()

    xin = nc.dram_tensor("xin", [NCHUNK, P, F6], BF16, kind="ExternalInput")
    yout = nc.dram_tensor("yout", [NCHUNK, P, F6], BF16, kind="ExternalOutput")

    V, G, S = nc.vector, nc.gpsimd, nc.scalar

    with tile.TileContext(nc) as tc:
        with tc.tile_pool(name="mp", bufs=1) as pool:

            def T(units, name, ci, dtype=BF16):
                tag = f"{name}{ci}"
                return pool.tile([P, units * FD], dtype, name=tag, tag=tag, bufs=1)

            def r3(ap):
                return ap.rearrange("p (c f) -> p c f", c=3)

            def r2_(ap):
                return ap.rearrange("p (c f) -> p c f", c=2)

            def mk_bias(name, probes, vals):
                """[P,1] bias-constant tiles whose writes depend on `probes`.

                Values are exact; the dependency orders a phase's ACT ops
                after all probes' producers, so the activation-table fixpoint
                sees clean one-table runs."""
                pb = pool.tile([P, 1], F32, name=f"pb_{name}", tag=f"pb_{name}")
                V.tensor_tensor(pb[:], probes[0], probes[1], OP.add)
                for pr in probes[2:]:
                    V.tensor_tensor(pb[:], pb[:], pr, OP.add)
                bt = pool.tile([P, len(vals)], F32, name=f"bt_{name}", tag=f"bt_{name}")
                out = {}
                for i, v in enumerate(vals):
                    V.tensor_scalar(bt[:, i:i + 1], pb[:], 0.0, float(v),
                                    OP.mult, OP.add)
                    out[v] = bt[:, i:i + 1]
                return out

            def phaseA(ci):
                t = {}
                xin_t = T(6, "xin", ci)
                t["xin"] = xin_t
                if ci == 0:
                    for u0, u1 in ((0, 2), (2, 4), (4, 6)):
                        srcp = bass.AP(xin, ci * P * F6 + u0 * FD,
                                       [[F6, P], [1, (u1 - u0) * FD]])
                        nc.sync.dma_start(xin_t[:, u0 * FD:u1 * FD], srcp)
                else:
                    src_adf = bass.AP(xin, ci * P * F6, [[F6, P], [1, 3 * FD]])
                    src_bce = bass.AP(xin, ci * P * F6 + 3 * FD, [[F6, P], [1, 3 * FD]])
                    nc.sync.dma_start(xin_t[:, 0:3 * FD], src_adf)
                    nc.sync.dma_start(xin_t[:, 3 * FD:6 * FD], src_bce)
                xr = xin_t[:]
                b_ = xr[:, 3 * FD:4 * FD]
                c_ = xr[:, 4 * FD:5 * FD]
                e_ = xr[:, 5 * FD:6 * FD]
                adf = r3(xr[:, 0:3 * FD])
                bce = xr[:, 3 * FD:6 * FD]

                q = T(1, "q", ci)
                V.tensor_tensor(q[:], xr[:, 0:FD], xr[:, FD:2 * FD], OP.add)
                V.tensor_tensor(q[:], q[:], xr[:, 2 * FD:3 * FD], OP.add)
                V.tensor_scalar(q[:], q[:], 1.0 / 3.0, None, OP.mult)
                t["q"] = q
                D3 = T(3, "D3", ci)
                V.tensor_tensor(r3(D3[:]), adf, _b(q[:], 3), OP.subtract)
                t["D3"] = D3

                SQ = T(6, "SQ", ci)
                S.activation(SQ[:, 0:3 * FD], D3[:], AF.Square)
                S.activation(SQ[:, 3 * FD:6 * FD], bce, AF.Square)
                t["SQ"] = SQ
                sq6 = SQ[:].rearrange("p (c f) -> p c f", c=6)
                sqo_rev = r3(SQ[:, 3 * FD:6 * FD])[:, ::-1, :]

                # pair-reduce into SQ lanes [0,1] (D^2 lanes, dead after st):
                # (D2a,b2)+(D2d,c2) -> (0,1); + (D2f,e2) -> stsu
                V.tensor_tensor(sq6[:, 0:2, :], sq6[:, 0:4:3, :], sq6[:, 1:5:3, :], OP.add)
                stsu = T(2, "stsu", ci)
                V.tensor_tensor(r2_(stsu[:]), sq6[:, 0:2, :], sq6[:, 2:6:3, :], OP.add)
                st = stsu[:, 0:FD]
                su = stsu[:, FD:2 * FD]
                t["su"] = stsu
                # p2u = st + 2*su accumulated in-place into the st lane
                V.tensor_tensor(st, st, su, OP.add)
                V.tensor_tensor(st, st, su, OP.add)
                pt = T(1, "pt", ci)
                S.activation(pt[:], st, AF.Sqrt, scale=1.0 / 6.0, bias=1e-30)
                t["pt"] = pt

                # det block: tau into SQ[0:3] (D^2 lanes dead after stsu)
                tau = SQ[:, 0:3 * FD]
                V.tensor_tensor(r3(tau), r3(D3[:]), sqo_rev, OP.mult)
                V.tensor_tensor(SQ[:, 0:FD], SQ[:, 0:FD], SQ[:, FD:2 * FD], OP.add)
                V.tensor_tensor(SQ[:, FD:2 * FD], SQ[:, 0:FD], SQ[:, 2 * FD:3 * FD], OP.add)
                dets = SQ[:, FD:2 * FD]
                ad = T(1, "ad", ci)
                V.tensor_tensor(ad[:], D3[:, 0:FD], D3[:, FD:2 * FD], OP.mult)
                V.tensor_tensor(ad[:], ad[:], D3[:, 2 * FD:3 * FD], OP.mult)
                cross = T(3, "cross", ci)
                t["cross"] = cross
                V.tensor_tensor(cross[:, 2 * FD:3 * FD], b_, c_, OP.mult)
                bce2 = T(1, "bce2", ci)
                V.tensor_tensor(bce2[:], cross[:, 2 * FD:3 * FD], e_, OP.mult)
                V.tensor_scalar(bce2[:], bce2[:], 2.0, None, OP.mult)
                V.tensor_tensor(ad[:], ad[:], dets, OP.subtract)
                V.tensor_tensor(ad[:], ad[:], bce2[:], OP.add)   # ad holds det
                t["det"] = ad

                # Pool: cross01 = (c,b)*e ; w3 = su - (e2,c2,b2)
                cb = r3(xr[:, 3 * FD:6 * FD])[:, 1::-1, :]
                G.tensor_tensor(r2_(cross[:, 0:2 * FD]), cb,
                                e_.unsqueeze(1).broadcast_to((P, 2, FD)), OP.mult)
                w3 = T(3, "w3", ci)
                G.tensor_tensor(r3(w3[:]), _b(su, 3), sqo_rev, OP.subtract)
                t["w3"] = w3
                return t

            def phaseAtail(ci, t, gate=None):
                gb = gate if gate is not None else {}
                ip = T(1, "ip", ci, F32)
                from concourse.dve_ops import (RECIP_APPROX_FAST_CONSTS as _RC,
                                               RECIPROCAL_APPROX_FAST as _RAF)
                V._custom_dve(_RAF, out=ip[:], in0=t["pt"][:],
                              s0=_RC["s0"], s1=_RC["s1"], imm2=_RC["imm2"])
                # rr overwrites ip (in1 of the custom; elementwise-safe)
                V._custom_dve(DETC_CLAMP, out=ip[:], in0=t["det"][:], in1=ip[:],
                              s0=0.5, s1=-CL, imm2=CL)
                rr = ip
                r2v = T(1, "r2v", ci, F32)
                S.activation(r2v[:], rr[:], AF.Square)
                sqv = T(1, "sqv", ci, F32)
                S.activation(sqv[:], r2v[:], AF.Sqrt, scale=-1.0, bias=gb.get(1.0, 1.0))
                # tq -> overwrite r2v (dead)
                V._custom_dve(RECIP1_MUL, out=r2v[:], in0=sqv[:], in1=rr[:],
                              s0=_RC0, s1=_RC1)
                t["tq"] = r2v
                t["sqv"] = sqv
                t["at"] = rr   # rr dead after tq; reuse for arctan output

            def phaseB1(ci, t, gate=None):
                gb = gate if gate is not None else {}
                at = t["at"]
                S.activation(at[:], t["tq"][:], AF.Arctan, bias=gb.get(0.0, 0.0))
                sfcf = T(2, "sfcf", ci)
                S.activation(sfcf[:, 0:FD], at[:], AF.Sin, scale=-1.0 / 3.0,
                             bias=gb.get(PI6, PI6))
                S.activation(sfcf[:, FD:2 * FD], at[:], AF.Sin, scale=-1.0 / 3.0,
                             bias=gb.get(PI6 + math.pi / 2.0, PI6 + math.pi / 2.0))
                t["sfcf"] = sfcf
                pcps = T(2, "pcps", ci)
                V.tensor_tensor(r2_(pcps[:]),
                                t["pt"][:].unsqueeze(1).broadcast_to((P, 2, FD)),
                                r2_(sfcf[:]), OP.mult)
                ps = pcps[:, 0:FD]
                pc = pcps[:, FD:2 * FD]
                t["pcps"] = pcps
                # sfcf dead: reuse lanes for ps3, pc2
                ps3 = sfcf[:, 0:FD]
                pc2 = sfcf[:, FD:2 * FD]
                V.tensor_scalar(ps3, ps, S3, None, OP.mult)
                V.tensor_scalar(pc2, pc, 2.0, None, OP.mult)
                t["pc2h"] = sfcf
                uu = T(1, "uu", ci)
                V.tensor_tensor(uu[:], ps3, pc, OP.add)
                t["uu"] = uu

                # LD lanes into SQ: [l1 | l3 | l2 | d12 | d23 | d13]
                SQ = t["SQ"]
                LD = SQ[:]
                ld6 = LD.rearrange("p (c f) -> p c f", c=6)
                V.tensor_tensor(LD[:, 0:FD], t["q"][:], uu[:], OP.subtract)      # l1
                V.tensor_tensor(LD[:, FD:2 * FD], t["q"][:], pc2, OP.add)        # l3
                V.tensor_scalar(LD[:, 3 * FD:4 * FD], ps3, 2.0, TINY,
                                OP.mult, OP.max)                                 # d12
                V._custom_dve(SCALE_SUBMAX, out=LD[:, 4 * FD:5 * FD], in0=pc,
                              in1=uu[:], s0=4.0, s1=TINY)                        # d23
                # (l2, d13) = (l1, d12) + (d12, d23)
                V.tensor_tensor(ld6[:, 2:6:3, :], ld6[:, 0:4:3, :],
                                ld6[:, 3:5:1, :], OP.add)
                RL = T(4, "RL", ci)
                V._custom_dve(RECIP1, out=RL[:].rearrange("p (c f) -> p c f", c=4),
                              in0=LD[:, 2 * FD:6 * FD].rearrange("p (c f) -> p c f", c=4),
                              s0=_RC0, s1=_RC1)
                t["RL"] = RL
                U2 = T(2, "U2", ci)
                V.tensor_tensor(r2_(U2[:]), ld6[:, 3:5, :],
                                RL[:, 0:FD].unsqueeze(1).broadcast_to((P, 2, FD)),
                                OP.mult)
                t["U2"] = U2

            def phaseB2(ci, t, gate=None):
                gb = gate if gate is not None else {}
                G2 = T(2, "G2", ci)
                S.activation(G2[:, 0:FD], t["U2"][:, 0:FD], AF.Ln, scale=-1.0,
                             bias=gb.get(1.0, 1.0))
                S.activation(G2[:, FD:2 * FD], t["U2"][:, FD:2 * FD], AF.Ln,
                             bias=gb.get(1.0, 1.0))
                lg1 = T(1, "lg1", ci)
                S.activation(lg1[:], t["SQ"][:, 0:FD], AF.Ln, bias=gb.get(0.0, 0.0))
                t["lg1"] = lg1
                cf2 = T(2, "cf2", ci)
                V.tensor_tensor(r2_(cf2[:]), r2_(G2[:]),
                                t["RL"][:, FD:3 * FD].rearrange("p (c f) -> p c f", c=2),
                                OP.mult)
                cc2 = T(1, "cc2", ci)
                V.tensor_tensor(cc2[:], cf2[:, 0:FD], cf2[:, FD:2 * FD], OP.add)
                V.tensor_tensor(cc2[:], cc2[:], t["RL"][:, 3 * FD:4 * FD], OP.mult)
                # s0 -> G2[0] (dead), sigma -> G2[1]
                s0_ = G2[:, 0:FD]
                sigma = G2[:, FD:2 * FD]
                V._custom_dve(RECIP1_MUL, out=s0_, in0=cc2[:], in1=cf2[:, 0:FD],
                              s0=_RC0, s1=_RC1)
                pc2 = t["pc2h"][:, FD:2 * FD]
                V.tensor_tensor(sigma, pc2, s0_, OP.subtract)
                uu = t["uu"]
                # gamma = uu*cc2*(sigma-uu) + lg1  (e1 -> cf2[0], e2 -> cf2[1])
                e1 = cf2[:, 0:FD]
                e2 = cf2[:, FD:2 * FD]
                V.tensor_tensor(e1, sigma, uu[:], OP.subtract)
                V.tensor_tensor(e2, uu[:], cc2[:], OP.mult)
                V.tensor_tensor(e1, e1, e2, OP.mult)
                V.tensor_tensor(lg1[:], e1, lg1[:], OP.add)
                gamma = lg1
                t["cc2"] = cc2
                t["sigma"] = sigma
                t["gamma"] = gamma

            def phaseB2out(ci, t):
                cc2 = t["cc2"]
                sigma = t["sigma"]
                gamma = t["gamma"]
                D3 = t["D3"]
                SQ = t["SQ"]
                xr = t["xin"][:]
                # diag: zd scratch in SQ[0:3] (l1 consumed by lg1 already)
                zd = SQ[:, 0:3 * FD]
                V.tensor_tensor(r3(zd), r3(D3[:]), _b(sigma, 3), OP.add)
                V.tensor_tensor(zd, zd, D3[:], OP.mult)
                V.tensor_tensor(zd, zd, t["w3"][:], OP.add)
                # Yd into xin[0:3] (adf dead), gamma-add in place, then DMA
                yd = xr[:, 0:3 * FD]
                V.tensor_tensor(r3(yd), r3(zd), _b(cc2[:], 3), OP.mult)
                V.tensor_tensor(r3(yd), r3(yd), _b(gamma[:], 3), OP.add)
                dst_d = bass.AP(yout, ci * P * F6, [[F6, P], [1, 3 * FD]])
                nc.sync.dma_start(dst_d, yd)

                # offdiag: o chain in SQ[3:6] (gap lanes dead after RL/U2);
                # Pool normally, V for the final chunk to shorten the drain
                E = V if ci == NCHUNK - 1 else G
                bce = xr[:, 3 * FD:6 * FD]
                zo = SQ[:, 3 * FD:6 * FD]
                E.tensor_tensor(r3(zo), _b(sigma, 3), r3(D3[:])[:, ::-1, :],
                                OP.subtract)
                E.tensor_tensor(zo, bce, zo, OP.mult)
                E.tensor_tensor(zo, zo, t["cross"][:], OP.add)
                yo = xr[:, 3 * FD:6 * FD]
                V.tensor_tensor(r3(yo), r3(zo), _b(cc2[:], 3), OP.mult)
                dst_o = bass.AP(yout, ci * P * F6 + 3 * FD, [[F6, P], [1, 3 * FD]])
                nc.sync.dma_start(dst_o, yo)

            ts = [phaseA(ci) for ci in range(NCHUNK)]
            for ci in range(NCHUNK):
                phaseAtail(ci, ts[ci])
            g_trig01 = mk_bias("tr01", [ts[0]["sqv"][:, 0:1], ts[1]["sqv"][:, 0:1]],
                               [0.0, PI6, PI6 + math.pi / 2.0])
            phaseB1(0, ts[0], g_trig01)
            phaseB1(1, ts[1], g_trig01)
            g_trig23 = mk_bias("tr23", [ts[2]["sqv"][:, 0:1], ts[3]["sqv"][:, 0:1]],
                               [0.0, PI6, PI6 + math.pi / 2.0])
            phaseB1(2, ts[2], g_trig23)
            phaseB1(3, ts[3], g_trig23)
            g_ln01 = mk_bias("ln01", [ts[0]["sfcf"][:, 0:1], ts[1]["sfcf"][:, 0:1]],
                             [1.0, 0.0])
            phaseB2(0, ts[0], g_ln01)
            phaseB2(1, ts[1], g_ln01)
            g_ln23 = mk_bias("ln23", [ts[2]["sfcf"][:, 0:1], ts[3]["sfcf"][:, 0:1]],
                             [1.0, 0.0])
            phaseB2(2, ts[2], g_ln23)
            phaseB2(3, ts[3], g_ln23)
            for ci in range(NCHUNK):
                phaseB2out(ci, ts[ci])
    nc.finalize()
    return nc


def kernel(x):
    x = np.ascontiguousarray(np.asarray(x), dtype=np.float32)
    xf = x.reshape(B, 9, NV)
    sel = [0, 4, 8, 1, 2, 5]  # a d f b c e
    in_maps = []
    for k in range(NCORE):
        sh = xf[:, sel, k * VPC:(k + 1) * VPC]
        sh = sh.reshape(B, 6, P, CPB, FD).transpose(0, 3, 2, 1, 4)
        arr = np.ascontiguousarray(sh).astype(ml_dtypes.bfloat16)
        in_maps.append({"xin": arr.reshape(NCHUNK, P, F6)})
    if "nc" not in _CACHE:
        _CACHE["nc"] = build()
    res = run_bass_kernel_spmd(_CACHE["nc"], in_maps, core_ids=list(range(NCORE)))
    out = np.empty((B, 9, NV), np.float32)
    for k in range(NCORE):
        yb = np.asarray(res.results[k]["yout"]).reshape(B, CPB, P, 6, FD)
        y6 = yb.astype(np.float32).transpose(0, 3, 2, 1, 4).reshape(B, 6, VPC)
        sl = slice(k * VPC, (k + 1) * VPC)
        out[:, 0, sl] = y6[:, 0]
        out[:, 4, sl] = y6[:, 1]
        out[:, 8, sl] = y6[:, 2]
        out[:, 1, sl] = y6[:, 3]
        out[:, 3, sl] = y6[:, 3]
        out[:, 2, sl] = y6[:, 4]
        out[:, 6, sl] = y6[:, 4]
        out[:, 5, sl] = y6[:, 5]
        out[:, 7, sl] = y6[:, 5]
    return out.reshape(x.shape)
